# revision 14
# baseline (speedup 1.0000x reference)
"""Trainium2 Bass kernel for BidPrefix: per-row cumprod + 3-point gather.

Reference semantics (per row b of inputs [B, 302]):
  rates = inputs[b, :300]; bid = int(inputs[b, 300]); mp = int(inputs[b, 301])
  cpz[k] = prod(rates[:k]) (cpz[0] = 1)
  out[b] = [cpz[bid], cpz[mp+1], cpz[mp]]

Strategy: pure data parallel over 8 NeuronCores (batch sharded, padded to
8*25088 rows), row = p*196 + t partition-major layout. Per core the work is
done by TWO hand-written multi-page custom DVE uop programs (registered at
import time), each streaming a whole GROUP of tiles in ONE instruction.

SBUF page layout per tile t (304 fp32 columns):
  [pad, pad, rates[0..299], bid, mp]   (input row DMAed into cols 2..303)

The uop FSM is purely COUNT-driven (every uop consumes src0, so the engine
never waits on an empty FIFO and no SUB_DIM_DONE semantics are needed):

  init(1, no consume) -> [pre] -> header(s) -> steady 150 -> steady 150
      -> boundary emit uop(s) -> loop (SRC_TENSOR_DONE ends after the
      last page's final consume)

  CPTAPB_ANT: fp32 MULTIPLY scan + one positional IS_EQ tap whose target
    (bid-1) is consumed per page from Src1 by a header uop and latched in
    a swap flop; the boundary uop emits the held tap.  Out [P, G] =
    cpz[bid] (0 when bid==0; fixed up with +[bid==0]).

  CPTAPM_ANT: same with TWO taps (mp-1, mp) latched in two swap flops,
    two hold registers, 2-cycle boundary emission.  Out [P, 2G] =
    (cpz[mp], cpz[mp+1]) per page.

The scan is sequential fp32 multiply, bit-exact with the reference's f32
cumprod. No accumulator reads, no scalar AP reads, no junk body writes:
DVE cost ~304 cycles per 128-row tile per op.  Input DMA is split across
the two HWDGE queues (sync + scalar engines); index prep and output
fixups run on gpsimd.
"""

import sys

if "/opt/trn_rl_repo" not in sys.path:
    sys.path.insert(0, "/opt/trn_rl_repo")

import numpy as np

S = 300
COLS = 302
PG = 304  # padded page width in SBUF
P = 128
NCORES = 8
TILES = 196
BPC = TILES * P  # 25088 rows per core
BTOT = 200000

TRACE = False
LAST_RESULTS = None

_OPS_REGISTERED = {}


class _HandDveOp:
    """Duck-types dve_ops.DveOp with a hand-written uop program."""

    def __init__(self, name, spec, subdim, uops_fn, rd1_en=True):
        self.name = name
        self.spec = spec
        self.subdim = subdim
        self._fn = uops_fn
        self._rd1 = rd1_en
        self._cache = {}

    def compile(self, ver):
        if ver not in self._cache:
            from concourse.dve_ops import get_dve_sub_opcode
            from concourse.dve_uop import DveOpSpec

            spec = DveOpSpec(
                name=self.name,
                opcode=get_dve_sub_opcode(self.name),
                uops=self._fn(),
                rd1_en=self._rd1,
            )
            spec.validate(ver)
            self._cache[ver] = spec
        return self._cache[ver]


def _uops_tapb():
    """One cpz tap per page; target from Src1. Count-driven FSM.

    Page = 304 src0 elements: [pad, pad(hdr), rates x300, bid, mp].
    Registers: pos@blk0, scan@blk2, h@blk4 (CURR feedback); swap t@blk1.
    uops: 0=init(no consume), 1=pre(1), 2=hdr(1+src1: latch t, h=0),
          3=steady(150), 4=steady(150), 5=b1(1: emit h, pos/scan reseed),
          6=tail(1: consume; loop or end).
    """
    from concourse.dve_uop import (
        DISABLE,
        ENABLE,
        AluInp,
        AluOp,
        DelayInp,
        InpSel,
        OutPath,
        OutSel,
        Trigger,
        UopConfig,
        UopDpConfig,
    )

    A = AluInp
    PD0, PD1 = A.PREV_DELAY_0, A.PREV_DELAY_1

    # --- uop0: init (1 cycle, NO consume): pos=-1, scan=1, h=0
    u0 = UopConfig()
    u0.enable_input(InpSel.ONE_F32, 1)  # delay0 = 1.0
    u0.enable_input(InpSel.ZERO, 2)  # delay1 = 0.0
    u0.repeat_count = 1
    u0.trigger = (Trigger.COUNT, Trigger.NONE, Trigger.NONE)
    u0.next_uop = (1, 0, 0)
    u0.datapath_config[0] = (
        UopDpConfig().enable_alu(AluOp.SUBTRACT, PD1, PD0).pass_through_delay(0, 1)
    )
    u0.datapath_config[1] = UopDpConfig().pass_through_delay(0, 1)
    u0.datapath_config[2] = (
        UopDpConfig().enable_alu(AluOp.BYPASS, PD0, PD0).pass_through_delay(1)
    )
    u0.datapath_config[3] = UopDpConfig().pass_through_delay(1)
    u0.datapath_config[4] = UopDpConfig().enable_alu(AluOp.BYPASS, PD1, PD1)

    # --- uop1: pre: consume 1 src0 pad, do nothing
    u1 = UopConfig()
    u1.require_inp0 = ENABLE
    u1.repeat_count = 1
    u1.trigger = (Trigger.COUNT, Trigger.NONE, Trigger.NONE)
    u1.next_uop = (2, 0, 0)

    # --- uop2: header: consume 1 src0 pad + 1 src1 (t=bid-1): latch t@1, h=0
    u2 = UopConfig()
    u2.enable_input(InpSel.SRC_1, 1)  # delay0 = t
    u2.enable_input(InpSel.ZERO, 2)  # delay1 = 0.0
    u2.require_inp0 = ENABLE
    u2.require_inp1 = ENABLE
    u2.repeat_count = 1
    u2.trigger = (Trigger.COUNT, Trigger.NONE, Trigger.NONE)
    u2.next_uop = (3, 0, 0)
    u2.datapath_config[0] = UopDpConfig().pass_through_delay(0, 1)
    cfg = UopDpConfig()
    cfg.op = AluOp.BYPASS
    cfg.alu_src0 = PD0
    cfg.alu_src1 = PD0
    cfg.swap_enable = ENABLE
    cfg.alu_out_enable = DISABLE
    cfg.pass_through_delay(1)
    u2.datapath_config[1] = cfg
    u2.datapath_config[2] = UopDpConfig().pass_through_delay(1)
    u2.datapath_config[3] = UopDpConfig().pass_through_delay(1)
    u2.datapath_config[4] = UopDpConfig().enable_alu(AluOp.BYPASS, PD1, PD1)

    # --- steady (x2): pos++, eq=(pos==t), scan*=r, m=eq*scan, h+=m
    def steady(nxt):
        u = UopConfig()
        u.enable_input(InpSel.ONE_F32, 1)  # delay0 = 1.0
        u.enable_input(InpSel.SRC_0, 2)  # delay1 = rate
        u.require_inp0 = ENABLE
        u.repeat_count = 150
        u.trigger = (Trigger.COUNT, Trigger.NONE, Trigger.NONE)
        u.next_uop = (nxt, 0, 0)
        u.datapath_config[0] = (
            UopDpConfig()
            .enable_alu(AluOp.ADD, A.CURR_ALU_OUT, PD0)
            .pass_through_delay(1)
        )
        u.datapath_config[1] = (
            UopDpConfig()
            .enable_alu(AluOp.IS_EQ, A.PREV_ALU_OUT, A.CURR_SWAP_OUT)
            .pass_through_delay(1)
        )
        u.datapath_config[2] = (
            UopDpConfig()
            .enable_alu(AluOp.MULTIPLY, A.CURR_ALU_OUT, PD1)
            .enable_delay_from_src(DelayInp.PREV_ALU_OUT, 2)
        )
        u.datapath_config[3] = UopDpConfig().enable_alu(
            AluOp.MULTIPLY, A.PREV_ALU_OUT, A.PREV_DELAY_2
        )
        u.datapath_config[4] = UopDpConfig().enable_alu(
            AluOp.ADD, A.CURR_ALU_OUT, A.PREV_ALU_OUT
        )
        return u

    u3 = steady(4)
    u4 = steady(5)

    # --- uop5: b1: consume 1 (bid col): emit h, reseed pos@0 scan@2
    u5 = UopConfig()
    u5.enable_input(InpSel.ONE_F32, 1)
    u5.enable_input(InpSel.ZERO, 2)
    u5.require_inp0 = ENABLE
    u5.repeat_count = 1
    u5.trigger = (Trigger.COUNT, Trigger.NONE, Trigger.NONE)
    u5.next_uop = (6, 0, 0)
    u5.datapath_config[0] = (
        UopDpConfig().enable_alu(AluOp.SUBTRACT, PD1, PD0).pass_through_delay(0)
    )
    u5.datapath_config[1] = UopDpConfig().pass_through_delay(0)
    u5.datapath_config[2] = UopDpConfig().enable_alu(AluOp.BYPASS, PD0, PD0)
    u5.datapath_config[4] = UopDpConfig().enable_alu(
        AluOp.BYPASS, A.CURR_ALU_OUT, A.CURR_ALU_OUT
    )
    u5.datapath_config[5] = UopDpConfig().enable_delay_from_src(
        DelayInp.PREV_ALU_OUT, 2
    )
    u5.datapath_config[6] = UopDpConfig().pass_through_delay(2)
    u5.datapath_config[7] = UopDpConfig().pass_through_delay(2)
    u5.enable_output(OutSel.DELAY_2, OutPath.WR0_LO)

    # --- uop6: tail: consume 1 (mp col); end after last page else loop
    u6 = UopConfig()
    u6.require_inp0 = ENABLE
    u6.repeat_count = 1
    u6.trigger = (Trigger.SRC_TENSOR_DONE, Trigger.COUNT, Trigger.NONE)
    u6.next_uop = (0, 1, 0)

    return [u0, u1, u2, u3, u4, u5, u6]


def _uops_tapm():
    """Two cpz taps per page (mp-1, mp from a 2-elem Src1 page).

    Registers: pos@0, eq1@1, eq2@2, scan@3, m1@4, h1@5, m2@6, h2@7.
    Swaps: t1@1, t2@2.
    uops: 0=init, 1=h_a(1+src1: latch t1@1, h2=0), 2=h_b(1+src1: latch
          t2@2), 3=steady(150), 4=steady(150), 5=b1(1: emit h1, pos/scan
          reseed), 6=b2(1: emit h2, h1=0; loop or end).
    """
    from concourse.dve_uop import (
        DISABLE,
        ENABLE,
        AluInp,
        AluOp,
        DelayInp,
        InpSel,
        OutPath,
        OutSel,
        Trigger,
        UopConfig,
        UopDpConfig,
    )

    A = AluInp
    PD0, PD1 = A.PREV_DELAY_0, A.PREV_DELAY_1

    # --- uop0: init: pos=-1@0, scan=1@3, h1=0@5, h2=0@7  (no consume)
    u0 = UopConfig()
    u0.enable_input(InpSel.ONE_F32, 1)
    u0.enable_input(InpSel.ZERO, 2)
    u0.repeat_count = 1
    u0.trigger = (Trigger.COUNT, Trigger.NONE, Trigger.NONE)
    u0.next_uop = (1, 0, 0)
    u0.datapath_config[0] = (
        UopDpConfig().enable_alu(AluOp.SUBTRACT, PD1, PD0).pass_through_delay(0, 1)
    )
    u0.datapath_config[1] = UopDpConfig().pass_through_delay(0, 1)
    u0.datapath_config[2] = UopDpConfig().pass_through_delay(0, 1)
    u0.datapath_config[3] = (
        UopDpConfig().enable_alu(AluOp.BYPASS, PD0, PD0).pass_through_delay(1)
    )
    u0.datapath_config[4] = UopDpConfig().pass_through_delay(1)
    u0.datapath_config[5] = (
        UopDpConfig().enable_alu(AluOp.BYPASS, PD1, PD1).pass_through_delay(1)
    )
    u0.datapath_config[6] = UopDpConfig().pass_through_delay(1)
    u0.datapath_config[7] = UopDpConfig().enable_alu(AluOp.BYPASS, PD1, PD1)

    # --- uop1: h_a: consume 1 src0 + 1 src1 (t1=mp-1): latch t1@1; h2=0@7
    u1 = UopConfig()
    u1.enable_input(InpSel.SRC_1, 1)
    u1.enable_input(InpSel.ZERO, 2)
    u1.require_inp0 = ENABLE
    u1.require_inp1 = ENABLE
    u1.repeat_count = 1
    u1.trigger = (Trigger.COUNT, Trigger.NONE, Trigger.NONE)
    u1.next_uop = (2, 0, 0)
    u1.datapath_config[0] = UopDpConfig().pass_through_delay(0, 1)
    cfg = UopDpConfig()
    cfg.op = AluOp.BYPASS
    cfg.alu_src0 = PD0
    cfg.alu_src1 = PD0
    cfg.swap_enable = ENABLE
    cfg.alu_out_enable = DISABLE
    cfg.pass_through_delay(1)
    u1.datapath_config[1] = cfg
    for k in range(2, 7):
        u1.datapath_config[k] = UopDpConfig().pass_through_delay(1)
    u1.datapath_config[7] = UopDpConfig().enable_alu(AluOp.BYPASS, PD1, PD1)

    # --- uop2: h_b: consume 1 src0 + 1 src1 (t2=mp): latch t2@2
    u2 = UopConfig()
    u2.enable_input(InpSel.SRC_1, 1)
    u2.require_inp0 = ENABLE
    u2.require_inp1 = ENABLE
    u2.repeat_count = 1
    u2.trigger = (Trigger.COUNT, Trigger.NONE, Trigger.NONE)
    u2.next_uop = (3, 0, 0)
    u2.datapath_config[0] = UopDpConfig().pass_through_delay(0)
    u2.datapath_config[1] = UopDpConfig().pass_through_delay(0)
    cfg = UopDpConfig()
    cfg.op = AluOp.BYPASS
    cfg.alu_src0 = PD0
    cfg.alu_src1 = PD0
    cfg.swap_enable = ENABLE
    cfg.alu_out_enable = DISABLE
    u2.datapath_config[2] = cfg

    # --- steady x2
    def steady(nxt):
        u = UopConfig()
        u.enable_input(InpSel.ONE_F32, 1)  # delay0 = 1.0
        u.enable_input(InpSel.SRC_0, 3)  # delay2 = rate
        u.require_inp0 = ENABLE
        u.repeat_count = 150
        u.trigger = (Trigger.COUNT, Trigger.NONE, Trigger.NONE)
        u.next_uop = (nxt, 0, 0)
        u.datapath_config[0] = (
            UopDpConfig()
            .enable_alu(AluOp.ADD, A.CURR_ALU_OUT, PD0)
            .pass_through_delay(2)
        )
        u.datapath_config[1] = (
            UopDpConfig()
            .enable_alu(AluOp.IS_EQ, A.PREV_ALU_OUT, A.CURR_SWAP_OUT)
            .enable_delay_from_src(DelayInp.PREV_ALU_OUT, 1)
            .pass_through_delay(2)
        )
        u.datapath_config[2] = (
            UopDpConfig()
            .enable_alu(AluOp.IS_EQ, PD1, A.CURR_SWAP_OUT)
            .enable_delay_from_src(DelayInp.PREV_ALU_OUT, 3)
            .pass_through_delay(2)
        )
        u.datapath_config[3] = (
            UopDpConfig()
            .enable_alu(AluOp.MULTIPLY, A.CURR_ALU_OUT, A.PREV_DELAY_2)
            .enable_delay_from_src(DelayInp.PREV_ALU_OUT, 4)
            .pass_through_delay(3)
        )
        u.datapath_config[4] = (
            UopDpConfig()
            .enable_alu(AluOp.MULTIPLY, A.PREV_ALU_OUT, A.PREV_DELAY_3)
            .enable_delay_from_src(DelayInp.PREV_ALU_OUT, 5)
            .pass_through_delay(4)
        )
        u.datapath_config[5] = (
            UopDpConfig()
            .enable_alu(AluOp.ADD, A.CURR_ALU_OUT, A.PREV_ALU_OUT)
            .pass_through_delay(4, 5)
        )
        u.datapath_config[6] = UopDpConfig().enable_alu(
            AluOp.MULTIPLY, A.PREV_DELAY_4, A.PREV_DELAY_5
        )
        u.datapath_config[7] = UopDpConfig().enable_alu(
            AluOp.ADD, A.CURR_ALU_OUT, A.PREV_ALU_OUT
        )
        return u

    u3 = steady(4)
    u4 = steady(5)

    # --- uop5: b1: consume 1 (bid col): emit h1 via delay2; reseed pos/scan
    u5 = UopConfig()
    u5.enable_input(InpSel.ONE_F32, 1)
    u5.enable_input(InpSel.ZERO, 2)
    u5.require_inp0 = ENABLE
    u5.repeat_count = 1
    u5.trigger = (Trigger.COUNT, Trigger.NONE, Trigger.NONE)
    u5.next_uop = (6, 0, 0)
    u5.datapath_config[0] = (
        UopDpConfig().enable_alu(AluOp.SUBTRACT, PD1, PD0).pass_through_delay(0)
    )
    u5.datapath_config[1] = UopDpConfig().pass_through_delay(0)
    u5.datapath_config[2] = UopDpConfig().pass_through_delay(0)
    u5.datapath_config[3] = UopDpConfig().enable_alu(AluOp.BYPASS, PD0, PD0)
    u5.datapath_config[5] = UopDpConfig().enable_alu(
        AluOp.BYPASS, A.CURR_ALU_OUT, A.CURR_ALU_OUT
    )
    u5.datapath_config[6] = UopDpConfig().enable_delay_from_src(
        DelayInp.PREV_ALU_OUT, 2
    )
    u5.datapath_config[7] = UopDpConfig().pass_through_delay(2)
    u5.enable_output(OutSel.DELAY_2, OutPath.WR0_LO)

    # --- uop6: b2: consume 1 (mp col): emit h2 (ALU_OUT@7); h1=0@5
    u6 = UopConfig()
    u6.enable_input(InpSel.ZERO, 2)
    u6.require_inp0 = ENABLE
    u6.repeat_count = 1
    u6.trigger = (Trigger.SRC_TENSOR_DONE, Trigger.COUNT, Trigger.NONE)
    u6.next_uop = (0, 1, 0)
    for k in range(0, 5):
        u6.datapath_config[k] = UopDpConfig().pass_through_delay(1)
    u6.datapath_config[5] = UopDpConfig().enable_alu(AluOp.BYPASS, PD1, PD1)
    u6.datapath_config[7] = UopDpConfig().enable_alu(
        AluOp.BYPASS, A.CURR_ALU_OUT, A.CURR_ALU_OUT
    )
    u6.enable_output(OutSel.ALU_OUT, OutPath.WR0_LO)

    return [u0, u1, u2, u3, u4, u5, u6]


def _split_pages(in0):
    """CoreSim in0 arrives as [P, F] flat (F = G*304) or [P, G, 304]."""
    x = np.asarray(in0, np.float32)
    if x.ndim == 2:
        g = x.shape[1] // PG
        x = x.reshape(x.shape[0], g, PG)
    return x[:, :, 2 : 2 + S]  # rates


def _ref_tapb(in0, in1, s0, s1, imm2):
    rates = _split_pages(in0)
    cp = np.cumprod(rates, axis=-1, dtype=np.float32)
    t = np.asarray(in1, np.float32).reshape(rates.shape[0], rates.shape[1])
    t = t.astype(np.int64)
    tc = np.clip(t, 0, S - 1)
    v = np.take_along_axis(cp, tc[..., None], axis=-1)[..., 0]
    out = np.where(t >= 0, v, np.float32(0.0)).astype(np.float32)
    return out


def _ref_tapm(in0, in1, s0, s1, imm2):
    rates = _split_pages(in0)
    cp = np.cumprod(rates, axis=-1, dtype=np.float32)
    idx = np.asarray(in1, np.float32).reshape(rates.shape[0], rates.shape[1], 2)
    out = np.zeros((rates.shape[0], rates.shape[1], 2), np.float32)
    for k in range(2):
        t = idx[..., k].astype(np.int64)
        tc = np.clip(t, 0, S - 1)
        v = np.take_along_axis(cp, tc[..., None], axis=-1)[..., 0]
        out[..., k] = np.where(t >= 0, v, np.float32(0.0))
    return out


def _get_ops():
    """Register the two hand-uop ops (idempotent). Returns (TAPB, TAPM)."""
    global _OPS_REGISTERED
    if _OPS_REGISTERED:
        return _OPS_REGISTERED["b"], _OPS_REGISTERED["m"]
    import concourse.dve_ops as dve_ops
    from concourse.dve_ops import OPS
    from concourse.dve_spec import AluOp as SAluOp
    from concourse.dve_spec import Spec, Src0, Src1, scan

    for name, uops_fn, ref in (
        ("CPTAPB_ANT", _uops_tapb, _ref_tapb),
        ("CPTAPM_ANT", _uops_tapm, _ref_tapm),
    ):
        if name not in dve_ops._SUB_OPCODE_FOR_NAME:
            spec = Spec(
                body=Src1 * scan(SAluOp.MULTIPLY, Src0),
                reference=ref,
            )
            op = _HandDveOp(name, spec, subdim=False, uops_fn=uops_fn)
            OPS.append(op)
            dve_ops._SUB_OPCODE_FOR_NAME[name] = (
                dve_ops._CUSTOM_DVE_ROW_BASE + len(OPS) - 1
            )
            dve_ops.CUSTOM_DVE_SPECS[name] = spec
            _OPS_REGISTERED["b" if name == "CPTAPB_ANT" else "m"] = op
        else:
            for op in OPS:
                if op.name == name:
                    _OPS_REGISTERED["b" if name == "CPTAPB_ANT" else "m"] = op
    return _OPS_REGISTERED["b"], _OPS_REGISTERED["m"]


def build_nc(tiles=TILES, group=28):
    import concourse.bacc as bacc
    import concourse.mybir as mybir
    from concourse import tile

    f32 = mybir.dt.float32
    A = mybir.AluOpType
    TAPB, TAPM = _get_ops()

    bpc = tiles * P
    if tiles == TILES:
        # small first groups (shorter pipeline fill: first DVE op starts
        # after a ~1/4-size DMA) and a small last group (shorter tail).
        group_sizes = [14, 21, 28, 28, 28, 28, 28, 14, 7]
    else:
        if tiles % group != 0:
            group = tiles
        group_sizes = [group] * (tiles // group)
    assert sum(group_sizes) == tiles

    nc = bacc.Bacc("TRN2", target_bir_lowering=False, debug=False)
    inp = nc.dram_tensor("inp", [bpc, COLS], f32, kind="ExternalInput")
    out = nc.dram_tensor("out", [bpc, 3], f32, kind="ExternalOutput")

    # row = p*tiles + t (partition-major) so group output DMAs coalesce
    vin = inp.ap().rearrange("(p t) c -> p t c", p=P)
    vout = out.ap().rearrange("(p t) k -> p t k", p=P)

    with tile.TileContext(nc) as tc:
        with (
            tc.tile_pool(name="raw", bufs=3) as rawp,
            tc.tile_pool(name="aux", bufs=3) as auxp,
        ):
            gmax = max(group_sizes)
            t0 = 0
            for gi, g in enumerate(group_sizes):
                rawf = rawp.tile([P, gmax, PG], f32, tag="raw")
                raw = rawf[:, 0:g, :]
                # input row -> cols 2..303 of each page; cols 0..1 are pads
                # (consumed but unused). Split across both HWDGE queues
                # (sync + scalar).
                nc.gpsimd.memset(raw[:, :, 0:2], 0.0)
                nc.sync.dma_start(raw[0:64, :, 2 : 2 + COLS], vin[0:64, t0 : t0 + g, :])
                nc.scalar.dma_start(
                    raw[64:128, :, 2 : 2 + COLS], vin[64:128, t0 : t0 + g, :]
                )

                bidc = raw[:, :, 302:303]
                mpc = raw[:, :, 303:304]

                # per-page tap targets, flat 1D src1 streams (gpsimd)
                in1b_f = auxp.tile([P, gmax, 1], f32, tag="in1b")
                in1b = in1b_f[:, 0:g, :]
                nc.gpsimd.tensor_scalar(in1b, bidc, -1.0, None, A.add)
                in1m_f = auxp.tile([P, gmax, 2], f32, tag="in1m")
                in1m = in1m_f[:, 0:g, :]
                nc.gpsimd.tensor_scalar(in1m[:, :, 0:1], mpc, -1.0, None, A.add)
                nc.gpsimd.tensor_scalar(in1m[:, :, 1:2], mpc, 0.0, None, A.add)

                hb_f = auxp.tile([P, gmax, 1], f32, tag="hb")
                hb = hb_f[:, 0:g, :]
                nc.vector._custom_dve(
                    TAPB,
                    out=hb.rearrange("p s k -> p (s k)"),
                    in0=raw,
                    in1=in1b.rearrange("p s k -> p (s k)"),
                )
                hm_f = auxp.tile([P, gmax, 2], f32, tag="hm")
                hm = hm_f[:, 0:g, :]
                nc.vector._custom_dve(
                    TAPM,
                    out=hm.rearrange("p s k -> p (s k)"),
                    in0=raw,
                    in1=in1m.rearrange("p s k -> p (s k)"),
                )

                # fixups (+1 where idx==0) and assembly of [P, g, 3]
                e0_f = auxp.tile([P, gmax, 1], f32, tag="e0")
                e0 = e0_f[:, 0:g, :]
                nc.gpsimd.tensor_scalar(e0, bidc, 0.0, None, A.is_equal)
                e1_f = auxp.tile([P, gmax, 1], f32, tag="e1")
                e1 = e1_f[:, 0:g, :]
                nc.gpsimd.tensor_scalar(e1, mpc, 0.0, None, A.is_equal)

                res_f = auxp.tile([P, gmax, 3], f32, tag="res")
                res = res_f[:, 0:g, :]
                nc.gpsimd.tensor_add(res[:, :, 0:1], hb, e0)
                nc.gpsimd.tensor_copy(res[:, :, 1:2], hm[:, :, 1:2])
                nc.gpsimd.tensor_add(res[:, :, 2:3], hm[:, :, 0:1], e1)

                nc.sync.dma_start(vout[:, t0 : t0 + g, :], res)
                t0 += g

    nc.compile()
    return nc


_NC_CACHE = {}


def _get_nc():
    key = (TILES, 28)
    if key not in _NC_CACHE:
        _NC_CACHE[key] = build_nc()
    return _NC_CACHE[key]


def kernel(inputs):
    global LAST_RESULTS
    x = np.ascontiguousarray(np.asarray(inputs), dtype=np.float32)
    assert x.shape == (BTOT, COLS), x.shape

    npad = BPC * NCORES - BTOT
    padrows = np.zeros((npad, COLS), dtype=np.float32)
    padrows[:, :S] = 1.0
    xp = np.concatenate([x, padrows], axis=0)
    shards = xp.reshape(NCORES, BPC, COLS)

    in_maps = [{"inp": np.ascontiguousarray(shards[c])} for c in range(NCORES)]

    nc = _get_nc()
    from concourse.bass_utils import run_bass_kernel_spmd

    r = run_bass_kernel_spmd(
        nc, in_maps, core_ids=list(range(NCORES)), trace=TRACE
    )
    LAST_RESULTS = r
    y = np.concatenate([r.results[c]["out"] for c in range(NCORES)], axis=0)
    return np.ascontiguousarray(y[:BTOT]).astype(np.float32)


# revision 15
# speedup vs baseline: 1.1545x; 1.1545x over previous
"""Trainium2 Bass kernel for BidPrefix: per-row cumprod + 3-point gather.

Reference semantics (per row b of inputs [B, 302]):
  rates = inputs[b, :300]; bid = int(inputs[b, 300]); mp = int(inputs[b, 301])
  cpz[k] = prod(rates[:k]) (cpz[0] = 1)
  out[b] = [cpz[bid], cpz[mp+1], cpz[mp]]

Strategy: pure data parallel over 8 NeuronCores (batch sharded, padded to
8*25088 rows), row = p*196 + t partition-major layout. Per core the work is
done by TWO hand-written multi-page custom DVE uop programs (registered at
import time), each streaming a whole GROUP of tiles in ONE instruction.

SBUF page layout per tile t (304 fp32 columns):
  [pad, pad, rates[0..299], bid, mp]   (input row DMAed into cols 2..303)

The uop FSM is purely COUNT-driven (every uop consumes src0, so the engine
never waits on an empty FIFO and no SUB_DIM_DONE semantics are needed):

  init(1, no consume) -> [pre] -> header(s) -> steady 150 -> steady 150
      -> boundary emit uop(s) -> loop (SRC_TENSOR_DONE ends after the
      last page's final consume)

  CPTAPB_ANT: fp32 MULTIPLY scan + one positional IS_EQ tap whose target
    (bid-1) is consumed per page from Src1 by a header uop and latched in
    a swap flop; the boundary uop emits the held tap.  Out [P, G] =
    cpz[bid] (0 when bid==0; fixed up with +[bid==0]).

  CPTAPM_ANT: same with TWO taps (mp-1, mp) latched in two swap flops,
    two hold registers, 2-cycle boundary emission.  Out [P, 2G] =
    (cpz[mp], cpz[mp+1]) per page.

The scan is sequential fp32 multiply, bit-exact with the reference's f32
cumprod. No accumulator reads, no scalar AP reads, no junk body writes:
DVE cost ~304 cycles per 128-row tile per op.  Input DMA is split across
the two HWDGE queues (sync + scalar engines); index prep and output
fixups run on gpsimd.
"""

import sys

if "/opt/trn_rl_repo" not in sys.path:
    sys.path.insert(0, "/opt/trn_rl_repo")

import numpy as np

S = 300
COLS = 302
PG = 304  # padded page width in SBUF
P = 128
NCORES = 8
TILES = 196
BPC = TILES * P  # 25088 rows per core
BTOT = 200000

TRACE = False
LAST_RESULTS = None

_OPS_REGISTERED = {}


class _HandDveOp:
    """Duck-types dve_ops.DveOp with a hand-written uop program."""

    def __init__(self, name, spec, subdim, uops_fn, rd1_en=True):
        self.name = name
        self.spec = spec
        self.subdim = subdim
        self._fn = uops_fn
        self._rd1 = rd1_en
        self._cache = {}

    def compile(self, ver):
        if ver not in self._cache:
            from concourse.dve_ops import get_dve_sub_opcode
            from concourse.dve_uop import DveOpSpec

            spec = DveOpSpec(
                name=self.name,
                opcode=get_dve_sub_opcode(self.name),
                uops=self._fn(),
                rd1_en=self._rd1,
            )
            spec.validate(ver)
            self._cache[ver] = spec
        return self._cache[ver]


def _uops_tapb():
    """One cpz tap per page; target from Src1. Count-driven FSM.

    Page = 304 src0 elements: [pad, pad(hdr), rates x300, bid, mp].
    Registers: pos@blk0, scan@blk2, h@blk4 (CURR feedback); swap t@blk1.
    uops: 0=init(no consume), 1=pre(1), 2=hdr(1+src1: latch t, h=0),
          3=steady(150), 4=steady(150), 5=b1(1: emit h, pos/scan reseed),
          6=tail(1: consume; loop or end).
    """
    from concourse.dve_uop import (
        DISABLE,
        ENABLE,
        AluInp,
        AluOp,
        DelayInp,
        InpSel,
        OutPath,
        OutSel,
        Trigger,
        UopConfig,
        UopDpConfig,
    )

    A = AluInp
    PD0, PD1 = A.PREV_DELAY_0, A.PREV_DELAY_1

    # --- uop0: init (1 cycle, NO consume): pos=-1, scan=1, h=0
    u0 = UopConfig()
    u0.enable_input(InpSel.ONE_F32, 1)  # delay0 = 1.0
    u0.enable_input(InpSel.ZERO, 2)  # delay1 = 0.0
    u0.repeat_count = 1
    u0.trigger = (Trigger.COUNT, Trigger.NONE, Trigger.NONE)
    u0.next_uop = (1, 0, 0)
    u0.datapath_config[0] = (
        UopDpConfig().enable_alu(AluOp.SUBTRACT, PD1, PD0).pass_through_delay(0, 1)
    )
    u0.datapath_config[1] = UopDpConfig().pass_through_delay(0, 1)
    u0.datapath_config[2] = (
        UopDpConfig().enable_alu(AluOp.BYPASS, PD0, PD0).pass_through_delay(1)
    )
    u0.datapath_config[3] = UopDpConfig().pass_through_delay(1)
    u0.datapath_config[4] = UopDpConfig().enable_alu(AluOp.BYPASS, PD1, PD1)

    # --- uop1: pre: consume 1 src0 pad, do nothing
    u1 = UopConfig()
    u1.require_inp0 = ENABLE
    u1.repeat_count = 1
    u1.trigger = (Trigger.COUNT, Trigger.NONE, Trigger.NONE)
    u1.next_uop = (2, 0, 0)

    # --- uop2: header: consume 1 src0 pad + 1 src1 (t=bid-1): latch t@1, h=0
    u2 = UopConfig()
    u2.enable_input(InpSel.SRC_1, 1)  # delay0 = t
    u2.enable_input(InpSel.ZERO, 2)  # delay1 = 0.0
    u2.require_inp0 = ENABLE
    u2.require_inp1 = ENABLE
    u2.repeat_count = 1
    u2.trigger = (Trigger.COUNT, Trigger.NONE, Trigger.NONE)
    u2.next_uop = (3, 0, 0)
    u2.datapath_config[0] = UopDpConfig().pass_through_delay(0, 1)
    cfg = UopDpConfig()
    cfg.op = AluOp.BYPASS
    cfg.alu_src0 = PD0
    cfg.alu_src1 = PD0
    cfg.swap_enable = ENABLE
    cfg.alu_out_enable = DISABLE
    cfg.pass_through_delay(1)
    u2.datapath_config[1] = cfg
    u2.datapath_config[2] = UopDpConfig().pass_through_delay(1)
    u2.datapath_config[3] = UopDpConfig().pass_through_delay(1)
    u2.datapath_config[4] = UopDpConfig().enable_alu(AluOp.BYPASS, PD1, PD1)

    # --- steady (x2): pos++, eq=(pos==t), scan*=r, m=eq*scan, h+=m
    def steady(nxt):
        u = UopConfig()
        u.enable_input(InpSel.ONE_F32, 1)  # delay0 = 1.0
        u.enable_input(InpSel.SRC_0, 2)  # delay1 = rate
        u.require_inp0 = ENABLE
        u.repeat_count = 150
        u.trigger = (Trigger.COUNT, Trigger.NONE, Trigger.NONE)
        u.next_uop = (nxt, 0, 0)
        u.datapath_config[0] = (
            UopDpConfig()
            .enable_alu(AluOp.ADD, A.CURR_ALU_OUT, PD0)
            .pass_through_delay(1)
        )
        u.datapath_config[1] = (
            UopDpConfig()
            .enable_alu(AluOp.IS_EQ, A.PREV_ALU_OUT, A.CURR_SWAP_OUT)
            .pass_through_delay(1)
        )
        u.datapath_config[2] = (
            UopDpConfig()
            .enable_alu(AluOp.MULTIPLY, A.CURR_ALU_OUT, PD1)
            .enable_delay_from_src(DelayInp.PREV_ALU_OUT, 2)
        )
        u.datapath_config[3] = UopDpConfig().enable_alu(
            AluOp.MULTIPLY, A.PREV_ALU_OUT, A.PREV_DELAY_2
        )
        u.datapath_config[4] = UopDpConfig().enable_alu(
            AluOp.ADD, A.CURR_ALU_OUT, A.PREV_ALU_OUT
        )
        return u

    u3 = steady(4)
    u4 = steady(5)

    # --- uop5: b1: consume 1 (bid col): emit h, reseed pos@0 scan@2
    u5 = UopConfig()
    u5.enable_input(InpSel.ONE_F32, 1)
    u5.enable_input(InpSel.ZERO, 2)
    u5.require_inp0 = ENABLE
    u5.repeat_count = 1
    u5.trigger = (Trigger.COUNT, Trigger.NONE, Trigger.NONE)
    u5.next_uop = (6, 0, 0)
    u5.datapath_config[0] = (
        UopDpConfig().enable_alu(AluOp.SUBTRACT, PD1, PD0).pass_through_delay(0)
    )
    u5.datapath_config[1] = UopDpConfig().pass_through_delay(0)
    u5.datapath_config[2] = UopDpConfig().enable_alu(AluOp.BYPASS, PD0, PD0)
    u5.datapath_config[4] = UopDpConfig().enable_alu(
        AluOp.BYPASS, A.CURR_ALU_OUT, A.CURR_ALU_OUT
    )
    u5.datapath_config[5] = UopDpConfig().enable_delay_from_src(
        DelayInp.PREV_ALU_OUT, 2
    )
    u5.datapath_config[6] = UopDpConfig().pass_through_delay(2)
    u5.datapath_config[7] = UopDpConfig().pass_through_delay(2)
    u5.enable_output(OutSel.DELAY_2, OutPath.WR0_LO)

    # --- uop6: tail: consume 1 (mp col); end after last page else loop
    u6 = UopConfig()
    u6.require_inp0 = ENABLE
    u6.repeat_count = 1
    u6.trigger = (Trigger.SRC_TENSOR_DONE, Trigger.COUNT, Trigger.NONE)
    u6.next_uop = (0, 1, 0)

    return [u0, u1, u2, u3, u4, u5, u6]


def _uops_tapm():
    """Two cpz taps per page (mp-1, mp from a 2-elem Src1 page).

    Registers: pos@0, eq1@1, eq2@2, scan@3, m1@4, h1@5, m2@6, h2@7.
    Swaps: t1@1, t2@2.
    uops: 0=init, 1=h_a(1+src1: latch t1@1, h2=0), 2=h_b(1+src1: latch
          t2@2), 3=steady(150), 4=steady(150), 5=b1(1: emit h1, pos/scan
          reseed), 6=b2(1: emit h2, h1=0; loop or end).
    """
    from concourse.dve_uop import (
        DISABLE,
        ENABLE,
        AluInp,
        AluOp,
        DelayInp,
        InpSel,
        OutPath,
        OutSel,
        Trigger,
        UopConfig,
        UopDpConfig,
    )

    A = AluInp
    PD0, PD1 = A.PREV_DELAY_0, A.PREV_DELAY_1

    # --- uop0: init: pos=-1@0, scan=1@3, h1=0@5, h2=0@7  (no consume)
    u0 = UopConfig()
    u0.enable_input(InpSel.ONE_F32, 1)
    u0.enable_input(InpSel.ZERO, 2)
    u0.repeat_count = 1
    u0.trigger = (Trigger.COUNT, Trigger.NONE, Trigger.NONE)
    u0.next_uop = (1, 0, 0)
    u0.datapath_config[0] = (
        UopDpConfig().enable_alu(AluOp.SUBTRACT, PD1, PD0).pass_through_delay(0, 1)
    )
    u0.datapath_config[1] = UopDpConfig().pass_through_delay(0, 1)
    u0.datapath_config[2] = UopDpConfig().pass_through_delay(0, 1)
    u0.datapath_config[3] = (
        UopDpConfig().enable_alu(AluOp.BYPASS, PD0, PD0).pass_through_delay(1)
    )
    u0.datapath_config[4] = UopDpConfig().pass_through_delay(1)
    u0.datapath_config[5] = (
        UopDpConfig().enable_alu(AluOp.BYPASS, PD1, PD1).pass_through_delay(1)
    )
    u0.datapath_config[6] = UopDpConfig().pass_through_delay(1)
    u0.datapath_config[7] = UopDpConfig().enable_alu(AluOp.BYPASS, PD1, PD1)

    # --- uop1: h_a: consume 1 src0 + 1 src1 (t1=mp-1): latch t1@1; h2=0@7
    u1 = UopConfig()
    u1.enable_input(InpSel.SRC_1, 1)
    u1.enable_input(InpSel.ZERO, 2)
    u1.require_inp0 = ENABLE
    u1.require_inp1 = ENABLE
    u1.repeat_count = 1
    u1.trigger = (Trigger.COUNT, Trigger.NONE, Trigger.NONE)
    u1.next_uop = (2, 0, 0)
    u1.datapath_config[0] = UopDpConfig().pass_through_delay(0, 1)
    cfg = UopDpConfig()
    cfg.op = AluOp.BYPASS
    cfg.alu_src0 = PD0
    cfg.alu_src1 = PD0
    cfg.swap_enable = ENABLE
    cfg.alu_out_enable = DISABLE
    cfg.pass_through_delay(1)
    u1.datapath_config[1] = cfg
    for k in range(2, 7):
        u1.datapath_config[k] = UopDpConfig().pass_through_delay(1)
    u1.datapath_config[7] = UopDpConfig().enable_alu(AluOp.BYPASS, PD1, PD1)

    # --- uop2: h_b: consume 1 src0 + 1 src1 (t2=mp): latch t2@2
    u2 = UopConfig()
    u2.enable_input(InpSel.SRC_1, 1)
    u2.require_inp0 = ENABLE
    u2.require_inp1 = ENABLE
    u2.repeat_count = 1
    u2.trigger = (Trigger.COUNT, Trigger.NONE, Trigger.NONE)
    u2.next_uop = (3, 0, 0)
    u2.datapath_config[0] = UopDpConfig().pass_through_delay(0)
    u2.datapath_config[1] = UopDpConfig().pass_through_delay(0)
    cfg = UopDpConfig()
    cfg.op = AluOp.BYPASS
    cfg.alu_src0 = PD0
    cfg.alu_src1 = PD0
    cfg.swap_enable = ENABLE
    cfg.alu_out_enable = DISABLE
    u2.datapath_config[2] = cfg

    # --- steady x2
    def steady(nxt):
        u = UopConfig()
        u.enable_input(InpSel.ONE_F32, 1)  # delay0 = 1.0
        u.enable_input(InpSel.SRC_0, 3)  # delay2 = rate
        u.require_inp0 = ENABLE
        u.repeat_count = 150
        u.trigger = (Trigger.COUNT, Trigger.NONE, Trigger.NONE)
        u.next_uop = (nxt, 0, 0)
        u.datapath_config[0] = (
            UopDpConfig()
            .enable_alu(AluOp.ADD, A.CURR_ALU_OUT, PD0)
            .pass_through_delay(2)
        )
        u.datapath_config[1] = (
            UopDpConfig()
            .enable_alu(AluOp.IS_EQ, A.PREV_ALU_OUT, A.CURR_SWAP_OUT)
            .enable_delay_from_src(DelayInp.PREV_ALU_OUT, 1)
            .pass_through_delay(2)
        )
        u.datapath_config[2] = (
            UopDpConfig()
            .enable_alu(AluOp.IS_EQ, PD1, A.CURR_SWAP_OUT)
            .enable_delay_from_src(DelayInp.PREV_ALU_OUT, 3)
            .pass_through_delay(2)
        )
        u.datapath_config[3] = (
            UopDpConfig()
            .enable_alu(AluOp.MULTIPLY, A.CURR_ALU_OUT, A.PREV_DELAY_2)
            .enable_delay_from_src(DelayInp.PREV_ALU_OUT, 4)
            .pass_through_delay(3)
        )
        u.datapath_config[4] = (
            UopDpConfig()
            .enable_alu(AluOp.MULTIPLY, A.PREV_ALU_OUT, A.PREV_DELAY_3)
            .enable_delay_from_src(DelayInp.PREV_ALU_OUT, 5)
            .pass_through_delay(4)
        )
        u.datapath_config[5] = (
            UopDpConfig()
            .enable_alu(AluOp.ADD, A.CURR_ALU_OUT, A.PREV_ALU_OUT)
            .pass_through_delay(4, 5)
        )
        u.datapath_config[6] = UopDpConfig().enable_alu(
            AluOp.MULTIPLY, A.PREV_DELAY_4, A.PREV_DELAY_5
        )
        u.datapath_config[7] = UopDpConfig().enable_alu(
            AluOp.ADD, A.CURR_ALU_OUT, A.PREV_ALU_OUT
        )
        return u

    u3 = steady(4)
    u4 = steady(5)

    # --- uop5: b1: consume 1 (bid col): emit h1 via delay2; reseed pos/scan
    u5 = UopConfig()
    u5.enable_input(InpSel.ONE_F32, 1)
    u5.enable_input(InpSel.ZERO, 2)
    u5.require_inp0 = ENABLE
    u5.repeat_count = 1
    u5.trigger = (Trigger.COUNT, Trigger.NONE, Trigger.NONE)
    u5.next_uop = (6, 0, 0)
    u5.datapath_config[0] = (
        UopDpConfig().enable_alu(AluOp.SUBTRACT, PD1, PD0).pass_through_delay(0)
    )
    u5.datapath_config[1] = UopDpConfig().pass_through_delay(0)
    u5.datapath_config[2] = UopDpConfig().pass_through_delay(0)
    u5.datapath_config[3] = UopDpConfig().enable_alu(AluOp.BYPASS, PD0, PD0)
    u5.datapath_config[5] = UopDpConfig().enable_alu(
        AluOp.BYPASS, A.CURR_ALU_OUT, A.CURR_ALU_OUT
    )
    u5.datapath_config[6] = UopDpConfig().enable_delay_from_src(
        DelayInp.PREV_ALU_OUT, 2
    )
    u5.datapath_config[7] = UopDpConfig().pass_through_delay(2)
    u5.enable_output(OutSel.DELAY_2, OutPath.WR0_LO)

    # --- uop6: b2: consume 1 (mp col): emit h2 (ALU_OUT@7); h1=0@5
    u6 = UopConfig()
    u6.enable_input(InpSel.ZERO, 2)
    u6.require_inp0 = ENABLE
    u6.repeat_count = 1
    u6.trigger = (Trigger.SRC_TENSOR_DONE, Trigger.COUNT, Trigger.NONE)
    u6.next_uop = (0, 1, 0)
    for k in range(0, 5):
        u6.datapath_config[k] = UopDpConfig().pass_through_delay(1)
    u6.datapath_config[5] = UopDpConfig().enable_alu(AluOp.BYPASS, PD1, PD1)
    u6.datapath_config[7] = UopDpConfig().enable_alu(
        AluOp.BYPASS, A.CURR_ALU_OUT, A.CURR_ALU_OUT
    )
    u6.enable_output(OutSel.ALU_OUT, OutPath.WR0_LO)

    return [u0, u1, u2, u3, u4, u5, u6]


def _split_pages(in0):
    """CoreSim in0 arrives as [P, F] flat (F = G*304) or [P, G, 304]."""
    x = np.asarray(in0, np.float32)
    if x.ndim == 2:
        g = x.shape[1] // PG
        x = x.reshape(x.shape[0], g, PG)
    return x[:, :, 2 : 2 + S]  # rates


def _ref_tapb(in0, in1, s0, s1, imm2):
    rates = _split_pages(in0)
    cp = np.cumprod(rates, axis=-1, dtype=np.float32)
    t = np.asarray(in1, np.float32).reshape(rates.shape[0], rates.shape[1])
    t = t.astype(np.int64)
    tc = np.clip(t, 0, S - 1)
    v = np.take_along_axis(cp, tc[..., None], axis=-1)[..., 0]
    out = np.where(t >= 0, v, np.float32(0.0)).astype(np.float32)
    return out


def _ref_tapm(in0, in1, s0, s1, imm2):
    rates = _split_pages(in0)
    cp = np.cumprod(rates, axis=-1, dtype=np.float32)
    idx = np.asarray(in1, np.float32).reshape(rates.shape[0], rates.shape[1], 2)
    out = np.zeros((rates.shape[0], rates.shape[1], 2), np.float32)
    for k in range(2):
        t = idx[..., k].astype(np.int64)
        tc = np.clip(t, 0, S - 1)
        v = np.take_along_axis(cp, tc[..., None], axis=-1)[..., 0]
        out[..., k] = np.where(t >= 0, v, np.float32(0.0))
    return out


def _get_ops():
    """Register the two hand-uop ops (idempotent). Returns (TAPB, TAPM)."""
    global _OPS_REGISTERED
    if _OPS_REGISTERED:
        return _OPS_REGISTERED["b"], _OPS_REGISTERED["m"]
    import concourse.dve_ops as dve_ops
    from concourse.dve_ops import OPS
    from concourse.dve_spec import AluOp as SAluOp
    from concourse.dve_spec import Spec, Src0, Src1, scan

    for name, uops_fn, ref in (
        ("CPTAPB_ANT", _uops_tapb, _ref_tapb),
        ("CPTAPM_ANT", _uops_tapm, _ref_tapm),
    ):
        if name not in dve_ops._SUB_OPCODE_FOR_NAME:
            spec = Spec(
                body=Src1 * scan(SAluOp.MULTIPLY, Src0),
                reference=ref,
            )
            op = _HandDveOp(name, spec, subdim=False, uops_fn=uops_fn)
            OPS.append(op)
            dve_ops._SUB_OPCODE_FOR_NAME[name] = (
                dve_ops._CUSTOM_DVE_ROW_BASE + len(OPS) - 1
            )
            dve_ops.CUSTOM_DVE_SPECS[name] = spec
            _OPS_REGISTERED["b" if name == "CPTAPB_ANT" else "m"] = op
        else:
            for op in OPS:
                if op.name == name:
                    _OPS_REGISTERED["b" if name == "CPTAPB_ANT" else "m"] = op
    return _OPS_REGISTERED["b"], _OPS_REGISTERED["m"]


def build_nc(tiles=TILES, group=28):
    import concourse.bacc as bacc
    import concourse.mybir as mybir
    from concourse import tile

    f32 = mybir.dt.float32
    A = mybir.AluOpType
    TAPB, TAPM = _get_ops()

    bpc = tiles * P
    if tiles == TILES:
        # small first groups (shorter pipeline fill: first DVE op starts
        # after a ~1/4-size DMA) and a small last group (shorter tail).
        group_sizes = [14, 28, 28, 28, 28, 28, 28, 14]
    else:
        if tiles % group != 0:
            group = tiles
        group_sizes = [group] * (tiles // group)
    assert sum(group_sizes) == tiles

    nc = bacc.Bacc("TRN2", target_bir_lowering=False, debug=False)
    inp = nc.dram_tensor("inp", [bpc, COLS], f32, kind="ExternalInput")
    out = nc.dram_tensor("out", [bpc, 3], f32, kind="ExternalOutput")

    # row = p*tiles + t (partition-major) so group output DMAs coalesce
    vin = inp.ap().rearrange("(p t) c -> p t c", p=P)
    vout = out.ap().rearrange("(p t) k -> p t k", p=P)

    with tile.TileContext(nc) as tc:
        with (
            tc.tile_pool(name="raw", bufs=3) as rawp,
            tc.tile_pool(name="aux", bufs=3) as auxp,
        ):
            gmax = max(group_sizes)
            t0 = 0
            for gi, g in enumerate(group_sizes):
                rawf = rawp.tile([P, gmax, PG], f32, tag="raw")
                raw = rawf[:, 0:g, :]
                # input row -> cols 2..303 of each page; cols 0..1 are pads
                # (consumed but unused). Split across both HWDGE queues
                # (sync + scalar).
                nc.gpsimd.memset(raw[:, :, 0:2], 0.0)
                nc.sync.dma_start(raw[0:64, :, 2 : 2 + COLS], vin[0:64, t0 : t0 + g, :])
                nc.scalar.dma_start(
                    raw[64:128, :, 2 : 2 + COLS], vin[64:128, t0 : t0 + g, :]
                )

                bidc = raw[:, :, 302:303]
                mpc = raw[:, :, 303:304]

                # per-page tap targets, flat 1D src1 streams (gpsimd)
                in1b_f = auxp.tile([P, gmax, 1], f32, tag="in1b")
                in1b = in1b_f[:, 0:g, :]
                nc.gpsimd.tensor_scalar(in1b, bidc, -1.0, None, A.add)
                in1m_f = auxp.tile([P, gmax, 2], f32, tag="in1m")
                in1m = in1m_f[:, 0:g, :]
                nc.gpsimd.tensor_scalar(in1m[:, :, 0:1], mpc, -1.0, None, A.add)
                nc.gpsimd.tensor_scalar(in1m[:, :, 1:2], mpc, 0.0, None, A.add)

                hb_f = auxp.tile([P, gmax, 1], f32, tag="hb")
                hb = hb_f[:, 0:g, :]
                nc.vector._custom_dve(
                    TAPB,
                    out=hb.rearrange("p s k -> p (s k)"),
                    in0=raw,
                    in1=in1b.rearrange("p s k -> p (s k)"),
                )
                hm_f = auxp.tile([P, gmax, 2], f32, tag="hm")
                hm = hm_f[:, 0:g, :]
                nc.vector._custom_dve(
                    TAPM,
                    out=hm.rearrange("p s k -> p (s k)"),
                    in0=raw,
                    in1=in1m.rearrange("p s k -> p (s k)"),
                )

                # fixups (+1 where idx==0) and assembly of [P, g, 3]
                e0_f = auxp.tile([P, gmax, 1], f32, tag="e0")
                e0 = e0_f[:, 0:g, :]
                nc.gpsimd.tensor_scalar(e0, bidc, 0.0, None, A.is_equal)
                e1_f = auxp.tile([P, gmax, 1], f32, tag="e1")
                e1 = e1_f[:, 0:g, :]
                nc.gpsimd.tensor_scalar(e1, mpc, 0.0, None, A.is_equal)

                res_f = auxp.tile([P, gmax, 3], f32, tag="res")
                res = res_f[:, 0:g, :]
                nc.gpsimd.tensor_add(res[:, :, 0:1], hb, e0)
                nc.gpsimd.tensor_copy(res[:, :, 1:2], hm[:, :, 1:2])
                nc.gpsimd.tensor_add(res[:, :, 2:3], hm[:, :, 0:1], e1)

                nc.sync.dma_start(vout[:, t0 : t0 + g, :], res)
                t0 += g

    nc.compile()
    return nc


_NC_CACHE = {}


def _get_nc():
    key = (TILES, 28)
    if key not in _NC_CACHE:
        _NC_CACHE[key] = build_nc()
    return _NC_CACHE[key]


def kernel(inputs):
    global LAST_RESULTS
    x = np.ascontiguousarray(np.asarray(inputs), dtype=np.float32)
    assert x.shape == (BTOT, COLS), x.shape

    npad = BPC * NCORES - BTOT
    padrows = np.zeros((npad, COLS), dtype=np.float32)
    padrows[:, :S] = 1.0
    xp = np.concatenate([x, padrows], axis=0)
    shards = xp.reshape(NCORES, BPC, COLS)

    in_maps = [{"inp": np.ascontiguousarray(shards[c])} for c in range(NCORES)]

    nc = _get_nc()
    from concourse.bass_utils import run_bass_kernel_spmd

    r = run_bass_kernel_spmd(
        nc, in_maps, core_ids=list(range(NCORES)), trace=TRACE
    )
    LAST_RESULTS = r
    y = np.concatenate([r.results[c]["out"] for c in range(NCORES)], axis=0)
    return np.ascontiguousarray(y[:BTOT]).astype(np.float32)


# revision 17
# speedup vs baseline: 1.4762x; 1.2786x over previous
"""Trainium2 Bass kernel for BidPrefix: per-row cumprod + 3-point gather.

Reference semantics (per row b of inputs [B, 302]):
  rates = inputs[b, :300]; bid = int(inputs[b, 300]); mp = int(inputs[b, 301])
  cpz[k] = prod(rates[:k]) (cpz[0] = 1)
  out[b] = [cpz[bid], cpz[mp+1], cpz[mp]]

Strategy: pure data parallel over 8 NeuronCores (batch sharded, padded to
8*25088 rows), row = p*196 + t partition-major layout. Per core the work is
done by TWO hand-written multi-page custom DVE uop programs (registered at
import time), each streaming a whole GROUP of tiles in ONE instruction.

SBUF page layout per tile t (304 fp32 columns):
  [pad, pad, rates[0..299], bid, mp]   (input row DMAed into cols 2..303)

The uop FSM is purely COUNT-driven (every uop consumes src0, so the engine
never waits on an empty FIFO and no SUB_DIM_DONE semantics are needed):

  init(1, no consume) -> [pre] -> header(s) -> steady 150 -> steady 150
      -> boundary emit uop(s) -> loop (SRC_TENSOR_DONE ends after the
      last page's final consume)

  CPTAPB_ANT: fp32 MULTIPLY scan + one positional IS_EQ tap whose target
    (bid-1) is consumed per page from Src1 by a header uop and latched in
    a swap flop; the boundary uop emits the held tap.  Out [P, G] =
    cpz[bid] (0 when bid==0; fixed up with +[bid==0]).

  CPTAPM_ANT: same with TWO taps (mp-1, mp) latched in two swap flops,
    two hold registers, 2-cycle boundary emission.  Out [P, 2G] =
    (cpz[mp], cpz[mp+1]) per page.

The scan is sequential fp32 multiply, bit-exact with the reference's f32
cumprod. No accumulator reads, no scalar AP reads, no junk body writes:
DVE cost ~304 cycles per 128-row tile per op.  Input DMA is split across
the two HWDGE queues (sync + scalar engines); index prep and output
fixups run on gpsimd.
"""

import sys

if "/opt/trn_rl_repo" not in sys.path:
    sys.path.insert(0, "/opt/trn_rl_repo")

import numpy as np

S = 300
COLS = 302
PG = 305  # padded page width in SBUF: [pad, pad, rates x300, bid, mp, pad]
P = 128
NCORES = 8
TILES = 196
BPC = TILES * P  # 25088 rows per core
BTOT = 200000

TRACE = False
LAST_RESULTS = None

_OPS_REGISTERED = {}


class _HandDveOp:
    """Duck-types dve_ops.DveOp with a hand-written uop program."""

    def __init__(self, name, spec, subdim, uops_fn, rd1_en=True):
        self.name = name
        self.spec = spec
        self.subdim = subdim
        self._fn = uops_fn
        self._rd1 = rd1_en
        self._cache = {}

    def compile(self, ver):
        if ver not in self._cache:
            from concourse.dve_ops import get_dve_sub_opcode
            from concourse.dve_uop import DveOpSpec

            spec = DveOpSpec(
                name=self.name,
                opcode=get_dve_sub_opcode(self.name),
                uops=self._fn(),
                rd1_en=self._rd1,
            )
            spec.validate(ver)
            self._cache[ver] = spec
        return self._cache[ver]


def _uops_tapb():
    """One cpz tap per page; target from Src1. Count-driven FSM.

    Page = 304 src0 elements: [pad, pad(hdr), rates x300, bid, mp].
    Registers: pos@blk0, scan@blk2, h@blk4 (CURR feedback); swap t@blk1.
    uops: 0=init(no consume), 1=pre(1), 2=hdr(1+src1: latch t, h=0),
          3=steady(150), 4=steady(150), 5=b1(1: emit h, pos/scan reseed),
          6=tail(1: consume; loop or end).
    """
    from concourse.dve_uop import (
        DISABLE,
        ENABLE,
        AluInp,
        AluOp,
        DelayInp,
        InpSel,
        OutPath,
        OutSel,
        Trigger,
        UopConfig,
        UopDpConfig,
    )

    A = AluInp
    PD0, PD1 = A.PREV_DELAY_0, A.PREV_DELAY_1

    # --- uop0: init (1 cycle, NO consume): pos=-1, scan=1, h=0
    u0 = UopConfig()
    u0.enable_input(InpSel.ONE_F32, 1)  # delay0 = 1.0
    u0.enable_input(InpSel.ZERO, 2)  # delay1 = 0.0
    u0.repeat_count = 1
    u0.trigger = (Trigger.COUNT, Trigger.NONE, Trigger.NONE)
    u0.next_uop = (1, 0, 0)
    u0.datapath_config[0] = (
        UopDpConfig().enable_alu(AluOp.SUBTRACT, PD1, PD0).pass_through_delay(0, 1)
    )
    u0.datapath_config[1] = UopDpConfig().pass_through_delay(0, 1)
    u0.datapath_config[2] = (
        UopDpConfig().enable_alu(AluOp.BYPASS, PD0, PD0).pass_through_delay(1)
    )
    u0.datapath_config[3] = UopDpConfig().pass_through_delay(1)
    u0.datapath_config[4] = UopDpConfig().enable_alu(AluOp.BYPASS, PD1, PD1)

    # --- uop1: pre: consume 1 src0 pad, do nothing
    u1 = UopConfig()
    u1.require_inp0 = ENABLE
    u1.repeat_count = 1
    u1.trigger = (Trigger.COUNT, Trigger.NONE, Trigger.NONE)
    u1.next_uop = (2, 0, 0)

    # --- uop2: header: consume 1 src0 pad + 1 src1 (t=bid-1): latch t@1, h=0
    u2 = UopConfig()
    u2.enable_input(InpSel.SRC_1, 1)  # delay0 = t
    u2.enable_input(InpSel.ZERO, 2)  # delay1 = 0.0
    u2.require_inp0 = ENABLE
    u2.require_inp1 = ENABLE
    u2.repeat_count = 1
    u2.trigger = (Trigger.COUNT, Trigger.NONE, Trigger.NONE)
    u2.next_uop = (3, 0, 0)
    u2.datapath_config[0] = UopDpConfig().pass_through_delay(0, 1)
    cfg = UopDpConfig()
    cfg.op = AluOp.BYPASS
    cfg.alu_src0 = PD0
    cfg.alu_src1 = PD0
    cfg.swap_enable = ENABLE
    cfg.alu_out_enable = DISABLE
    cfg.pass_through_delay(1)
    u2.datapath_config[1] = cfg
    u2.datapath_config[2] = UopDpConfig().pass_through_delay(1)
    u2.datapath_config[3] = UopDpConfig().pass_through_delay(1)
    u2.datapath_config[4] = UopDpConfig().enable_alu(AluOp.BYPASS, PD1, PD1)

    # --- steady (x2): pos++, eq=(pos==t), scan*=r, m=eq*scan, h+=m
    def steady(nxt):
        u = UopConfig()
        u.enable_input(InpSel.ONE_F32, 1)  # delay0 = 1.0
        u.enable_input(InpSel.SRC_0, 2)  # delay1 = rate
        u.require_inp0 = ENABLE
        u.repeat_count = 150
        u.trigger = (Trigger.COUNT, Trigger.NONE, Trigger.NONE)
        u.next_uop = (nxt, 0, 0)
        u.datapath_config[0] = (
            UopDpConfig()
            .enable_alu(AluOp.ADD, A.CURR_ALU_OUT, PD0)
            .pass_through_delay(1)
        )
        u.datapath_config[1] = (
            UopDpConfig()
            .enable_alu(AluOp.IS_EQ, A.PREV_ALU_OUT, A.CURR_SWAP_OUT)
            .pass_through_delay(1)
        )
        u.datapath_config[2] = (
            UopDpConfig()
            .enable_alu(AluOp.MULTIPLY, A.CURR_ALU_OUT, PD1)
            .enable_delay_from_src(DelayInp.PREV_ALU_OUT, 2)
        )
        u.datapath_config[3] = UopDpConfig().enable_alu(
            AluOp.MULTIPLY, A.PREV_ALU_OUT, A.PREV_DELAY_2
        )
        u.datapath_config[4] = UopDpConfig().enable_alu(
            AluOp.ADD, A.CURR_ALU_OUT, A.PREV_ALU_OUT
        )
        return u

    u3 = steady(4)
    u4 = steady(5)

    # --- uop5: b1: consume 1 (bid col): emit h, reseed pos@0 scan@2
    u5 = UopConfig()
    u5.enable_input(InpSel.ONE_F32, 1)
    u5.enable_input(InpSel.ZERO, 2)
    u5.require_inp0 = ENABLE
    u5.repeat_count = 1
    u5.trigger = (Trigger.COUNT, Trigger.NONE, Trigger.NONE)
    u5.next_uop = (6, 0, 0)
    u5.datapath_config[0] = (
        UopDpConfig().enable_alu(AluOp.SUBTRACT, PD1, PD0).pass_through_delay(0)
    )
    u5.datapath_config[1] = UopDpConfig().pass_through_delay(0)
    u5.datapath_config[2] = UopDpConfig().enable_alu(AluOp.BYPASS, PD0, PD0)
    u5.datapath_config[4] = UopDpConfig().enable_alu(
        AluOp.BYPASS, A.CURR_ALU_OUT, A.CURR_ALU_OUT
    )
    u5.datapath_config[5] = UopDpConfig().enable_delay_from_src(
        DelayInp.PREV_ALU_OUT, 2
    )
    u5.datapath_config[6] = UopDpConfig().pass_through_delay(2)
    u5.datapath_config[7] = UopDpConfig().pass_through_delay(2)
    u5.enable_output(OutSel.DELAY_2, OutPath.WR0_LO)

    # --- uop6: tail: consume 1 (mp col); end after last page else loop
    u6 = UopConfig()
    u6.require_inp0 = ENABLE
    u6.repeat_count = 1
    u6.trigger = (Trigger.SRC_TENSOR_DONE, Trigger.COUNT, Trigger.NONE)
    u6.next_uop = (0, 1, 0)

    return [u0, u1, u2, u3, u4, u5, u6]


def _uops_tapm():
    """Two cpz taps per page (mp-1, mp from a 2-elem Src1 page).

    Registers: pos@0, eq1@1, eq2@2, scan@3, m1@4, h1@5, m2@6, h2@7.
    Swaps: t1@1, t2@2.
    uops: 0=init, 1=h_a(1+src1: latch t1@1, h2=0), 2=h_b(1+src1: latch
          t2@2), 3=steady(150), 4=steady(150), 5=b1(1: emit h1, pos/scan
          reseed), 6=b2(1: emit h2, h1=0; loop or end).
    """
    from concourse.dve_uop import (
        DISABLE,
        ENABLE,
        AluInp,
        AluOp,
        DelayInp,
        InpSel,
        OutPath,
        OutSel,
        Trigger,
        UopConfig,
        UopDpConfig,
    )

    A = AluInp
    PD0, PD1 = A.PREV_DELAY_0, A.PREV_DELAY_1

    # --- uop0: init: pos=-1@0, scan=1@3, h1=0@5, h2=0@7  (no consume)
    u0 = UopConfig()
    u0.enable_input(InpSel.ONE_F32, 1)
    u0.enable_input(InpSel.ZERO, 2)
    u0.repeat_count = 1
    u0.trigger = (Trigger.COUNT, Trigger.NONE, Trigger.NONE)
    u0.next_uop = (1, 0, 0)
    u0.datapath_config[0] = (
        UopDpConfig().enable_alu(AluOp.SUBTRACT, PD1, PD0).pass_through_delay(0, 1)
    )
    u0.datapath_config[1] = UopDpConfig().pass_through_delay(0, 1)
    u0.datapath_config[2] = UopDpConfig().pass_through_delay(0, 1)
    u0.datapath_config[3] = (
        UopDpConfig().enable_alu(AluOp.BYPASS, PD0, PD0).pass_through_delay(1)
    )
    u0.datapath_config[4] = UopDpConfig().pass_through_delay(1)
    u0.datapath_config[5] = (
        UopDpConfig().enable_alu(AluOp.BYPASS, PD1, PD1).pass_through_delay(1)
    )
    u0.datapath_config[6] = UopDpConfig().pass_through_delay(1)
    u0.datapath_config[7] = UopDpConfig().enable_alu(AluOp.BYPASS, PD1, PD1)

    # --- uop1: h_a: consume 1 src0 + 1 src1 (t1=mp-1): latch t1@1; h2=0@7
    u1 = UopConfig()
    u1.enable_input(InpSel.SRC_1, 1)
    u1.enable_input(InpSel.ZERO, 2)
    u1.require_inp0 = ENABLE
    u1.require_inp1 = ENABLE
    u1.repeat_count = 1
    u1.trigger = (Trigger.COUNT, Trigger.NONE, Trigger.NONE)
    u1.next_uop = (2, 0, 0)
    u1.datapath_config[0] = UopDpConfig().pass_through_delay(0, 1)
    cfg = UopDpConfig()
    cfg.op = AluOp.BYPASS
    cfg.alu_src0 = PD0
    cfg.alu_src1 = PD0
    cfg.swap_enable = ENABLE
    cfg.alu_out_enable = DISABLE
    cfg.pass_through_delay(1)
    u1.datapath_config[1] = cfg
    for k in range(2, 7):
        u1.datapath_config[k] = UopDpConfig().pass_through_delay(1)
    u1.datapath_config[7] = UopDpConfig().enable_alu(AluOp.BYPASS, PD1, PD1)

    # --- uop2: h_b: consume 1 src0 + 1 src1 (t2=mp): latch t2@2
    u2 = UopConfig()
    u2.enable_input(InpSel.SRC_1, 1)
    u2.require_inp0 = ENABLE
    u2.require_inp1 = ENABLE
    u2.repeat_count = 1
    u2.trigger = (Trigger.COUNT, Trigger.NONE, Trigger.NONE)
    u2.next_uop = (3, 0, 0)
    u2.datapath_config[0] = UopDpConfig().pass_through_delay(0)
    u2.datapath_config[1] = UopDpConfig().pass_through_delay(0)
    cfg = UopDpConfig()
    cfg.op = AluOp.BYPASS
    cfg.alu_src0 = PD0
    cfg.alu_src1 = PD0
    cfg.swap_enable = ENABLE
    cfg.alu_out_enable = DISABLE
    u2.datapath_config[2] = cfg

    # --- steady x2
    def steady(nxt):
        u = UopConfig()
        u.enable_input(InpSel.ONE_F32, 1)  # delay0 = 1.0
        u.enable_input(InpSel.SRC_0, 3)  # delay2 = rate
        u.require_inp0 = ENABLE
        u.repeat_count = 150
        u.trigger = (Trigger.COUNT, Trigger.NONE, Trigger.NONE)
        u.next_uop = (nxt, 0, 0)
        u.datapath_config[0] = (
            UopDpConfig()
            .enable_alu(AluOp.ADD, A.CURR_ALU_OUT, PD0)
            .pass_through_delay(2)
        )
        u.datapath_config[1] = (
            UopDpConfig()
            .enable_alu(AluOp.IS_EQ, A.PREV_ALU_OUT, A.CURR_SWAP_OUT)
            .enable_delay_from_src(DelayInp.PREV_ALU_OUT, 1)
            .pass_through_delay(2)
        )
        u.datapath_config[2] = (
            UopDpConfig()
            .enable_alu(AluOp.IS_EQ, PD1, A.CURR_SWAP_OUT)
            .enable_delay_from_src(DelayInp.PREV_ALU_OUT, 3)
            .pass_through_delay(2)
        )
        u.datapath_config[3] = (
            UopDpConfig()
            .enable_alu(AluOp.MULTIPLY, A.CURR_ALU_OUT, A.PREV_DELAY_2)
            .enable_delay_from_src(DelayInp.PREV_ALU_OUT, 4)
            .pass_through_delay(3)
        )
        u.datapath_config[4] = (
            UopDpConfig()
            .enable_alu(AluOp.MULTIPLY, A.PREV_ALU_OUT, A.PREV_DELAY_3)
            .enable_delay_from_src(DelayInp.PREV_ALU_OUT, 5)
            .pass_through_delay(4)
        )
        u.datapath_config[5] = (
            UopDpConfig()
            .enable_alu(AluOp.ADD, A.CURR_ALU_OUT, A.PREV_ALU_OUT)
            .pass_through_delay(4, 5)
        )
        u.datapath_config[6] = UopDpConfig().enable_alu(
            AluOp.MULTIPLY, A.PREV_DELAY_4, A.PREV_DELAY_5
        )
        u.datapath_config[7] = UopDpConfig().enable_alu(
            AluOp.ADD, A.CURR_ALU_OUT, A.PREV_ALU_OUT
        )
        return u

    u3 = steady(4)
    u4 = steady(5)

    # --- uop5: b1: consume 1 (bid col): emit h1 via delay2; reseed pos/scan
    u5 = UopConfig()
    u5.enable_input(InpSel.ONE_F32, 1)
    u5.enable_input(InpSel.ZERO, 2)
    u5.require_inp0 = ENABLE
    u5.repeat_count = 1
    u5.trigger = (Trigger.COUNT, Trigger.NONE, Trigger.NONE)
    u5.next_uop = (6, 0, 0)
    u5.datapath_config[0] = (
        UopDpConfig().enable_alu(AluOp.SUBTRACT, PD1, PD0).pass_through_delay(0)
    )
    u5.datapath_config[1] = UopDpConfig().pass_through_delay(0)
    u5.datapath_config[2] = UopDpConfig().pass_through_delay(0)
    u5.datapath_config[3] = UopDpConfig().enable_alu(AluOp.BYPASS, PD0, PD0)
    u5.datapath_config[5] = UopDpConfig().enable_alu(
        AluOp.BYPASS, A.CURR_ALU_OUT, A.CURR_ALU_OUT
    )
    u5.datapath_config[6] = UopDpConfig().enable_delay_from_src(
        DelayInp.PREV_ALU_OUT, 2
    )
    u5.datapath_config[7] = UopDpConfig().pass_through_delay(2)
    u5.enable_output(OutSel.DELAY_2, OutPath.WR0_LO)

    # --- uop6: b2: consume 1 (mp col): emit h2 (ALU_OUT@7); h1=0@5
    u6 = UopConfig()
    u6.enable_input(InpSel.ZERO, 2)
    u6.require_inp0 = ENABLE
    u6.repeat_count = 1
    u6.trigger = (Trigger.SRC_TENSOR_DONE, Trigger.COUNT, Trigger.NONE)
    u6.next_uop = (0, 1, 0)
    for k in range(0, 5):
        u6.datapath_config[k] = UopDpConfig().pass_through_delay(1)
    u6.datapath_config[5] = UopDpConfig().enable_alu(AluOp.BYPASS, PD1, PD1)
    u6.datapath_config[7] = UopDpConfig().enable_alu(
        AluOp.BYPASS, A.CURR_ALU_OUT, A.CURR_ALU_OUT
    )
    u6.enable_output(OutSel.ALU_OUT, OutPath.WR0_LO)

    return [u0, u1, u2, u3, u4, u5, u6]


def _split_pages(in0):
    """CoreSim in0 arrives as [P, F] flat (F = G*304) or [P, G, 304]."""
    x = np.asarray(in0, np.float32)
    if x.ndim == 2:
        g = x.shape[1] // PG
        x = x.reshape(x.shape[0], g, PG)
    return x[:, :, 2 : 2 + S]  # rates


def _ref_tapb(in0, in1, s0, s1, imm2):
    rates = _split_pages(in0)
    cp = np.cumprod(rates, axis=-1, dtype=np.float32)
    t = np.asarray(in1, np.float32).reshape(rates.shape[0], rates.shape[1])
    t = t.astype(np.int64)
    tc = np.clip(t, 0, S - 1)
    v = np.take_along_axis(cp, tc[..., None], axis=-1)[..., 0]
    out = np.where(t >= 0, v, np.float32(0.0)).astype(np.float32)
    return out


def _ref_tapm(in0, in1, s0, s1, imm2):
    rates = _split_pages(in0)
    cp = np.cumprod(rates, axis=-1, dtype=np.float32)
    idx = np.asarray(in1, np.float32).reshape(rates.shape[0], rates.shape[1], 2)
    out = np.zeros((rates.shape[0], rates.shape[1], 2), np.float32)
    for k in range(2):
        t = idx[..., k].astype(np.int64)
        tc = np.clip(t, 0, S - 1)
        v = np.take_along_axis(cp, tc[..., None], axis=-1)[..., 0]
        out[..., k] = np.where(t >= 0, v, np.float32(0.0))
    return out




def _uops_tap3():
    """ALL THREE taps in ONE pass via compare+select capture-holds.

    Page = 305 src0 elements: [pad, pad, rates x300, bid, mp, pad].
    h = (cond ? scan : h) captures the scan value at the last position
    where the prefix-condition holds; seeds of 1.0 make bid==0 / mp==0
    correct with no fixups.
      h1: cond = pos <  bid  -> cpz[bid]
      h2: cond = pos <= mp   -> cpz[mp+1]
      h3: cond = pos <  mp   -> cpz[mp]
    Registers: pos@0, scan@1, h1@3, h2@5, h3@7; swaps: bid@2, mp@4, mp@6.
    uops: 0=init, 1=hdr1(+src1 bid: latch@2, h3=1), 2=hdr2(+src1 mp:
          latch@4+@6), 3/4=steady(150 each), 5=b1(emit h1, pos/scan
          reseed), 6=b2(emit h2, h1=1), 7=b3(emit h3, h2=1; loop/end).
    """
    from concourse.dve_uop import (
        DISABLE,
        ENABLE,
        AluInp,
        AluOp,
        DelayInp,
        InpSel,
        OutPath,
        OutSel,
        Trigger,
        UopConfig,
        UopDpConfig,
    )

    A = AluInp
    PD0, PD1 = A.PREV_DELAY_0, A.PREV_DELAY_1
    PD2, PD3 = A.PREV_DELAY_2, A.PREV_DELAY_3

    # --- uop0: init (no consume): pos=-1@0, scan=1@1, h1/h2/h3=1@3/5/7
    u0 = UopConfig()
    u0.enable_input(InpSel.ONE_F32, 1)  # d0 = 1.0
    u0.enable_input(InpSel.ZERO, 2)  # d1 = 0.0
    u0.repeat_count = 1
    u0.trigger = (Trigger.COUNT, Trigger.NONE, Trigger.NONE)
    u0.next_uop = (1, 0, 0)
    u0.datapath_config[0] = (
        UopDpConfig().enable_alu(AluOp.SUBTRACT, PD1, PD0).pass_through_delay(0)
    )
    u0.datapath_config[1] = (
        UopDpConfig().enable_alu(AluOp.BYPASS, PD0, PD0).pass_through_delay(0)
    )
    u0.datapath_config[2] = UopDpConfig().pass_through_delay(0)
    u0.datapath_config[3] = (
        UopDpConfig().enable_alu(AluOp.BYPASS, PD0, PD0).pass_through_delay(0)
    )
    u0.datapath_config[4] = UopDpConfig().pass_through_delay(0)
    u0.datapath_config[5] = (
        UopDpConfig().enable_alu(AluOp.BYPASS, PD0, PD0).pass_through_delay(0)
    )
    u0.datapath_config[6] = UopDpConfig().pass_through_delay(0)
    u0.datapath_config[7] = UopDpConfig().enable_alu(AluOp.BYPASS, PD0, PD0)

    # --- uop1: hdr1: consume pad + src1(bid): latch bid@2; h3=1@7
    u1 = UopConfig()
    u1.enable_input(InpSel.SRC_1, 1)  # d0 = bid
    u1.enable_input(InpSel.ONE_F32, 2)  # d1 = 1.0
    u1.require_inp0 = ENABLE
    u1.require_inp1 = ENABLE
    u1.repeat_count = 1
    u1.trigger = (Trigger.COUNT, Trigger.NONE, Trigger.NONE)
    u1.next_uop = (2, 0, 0)
    u1.datapath_config[0] = UopDpConfig().pass_through_delay(0, 1)
    u1.datapath_config[1] = UopDpConfig().pass_through_delay(0, 1)
    cfg = UopDpConfig()
    cfg.op = AluOp.BYPASS
    cfg.alu_src0 = PD0
    cfg.alu_src1 = PD0
    cfg.swap_enable = ENABLE
    cfg.alu_out_enable = DISABLE
    cfg.pass_through_delay(1)
    u1.datapath_config[2] = cfg
    for k in range(3, 7):
        u1.datapath_config[k] = UopDpConfig().pass_through_delay(1)
    u1.datapath_config[7] = UopDpConfig().enable_alu(AluOp.BYPASS, PD1, PD1)

    # --- uop2: hdr2: consume pad + src1(mp): latch mp@4 AND mp@6
    u2 = UopConfig()
    u2.enable_input(InpSel.SRC_1, 1)  # d0 = mp
    u2.require_inp0 = ENABLE
    u2.require_inp1 = ENABLE
    u2.repeat_count = 1
    u2.trigger = (Trigger.COUNT, Trigger.NONE, Trigger.NONE)
    u2.next_uop = (3, 0, 0)
    for k in range(0, 4):
        u2.datapath_config[k] = UopDpConfig().pass_through_delay(0)
    cfg = UopDpConfig()
    cfg.op = AluOp.BYPASS
    cfg.alu_src0 = PD0
    cfg.alu_src1 = PD0
    cfg.swap_enable = ENABLE
    cfg.alu_out_enable = DISABLE
    cfg.pass_through_delay(0)
    u2.datapath_config[4] = cfg
    u2.datapath_config[5] = UopDpConfig().pass_through_delay(0)
    cfg = UopDpConfig()
    cfg.op = AluOp.BYPASS
    cfg.alu_src0 = PD0
    cfg.alu_src1 = PD0
    cfg.swap_enable = ENABLE
    cfg.alu_out_enable = DISABLE
    u2.datapath_config[6] = cfg

    # --- steady x2: pos++, scan*=, three cond/select capture-holds
    def steady(nxt):
        u = UopConfig()
        u.enable_input(InpSel.ONE_F32, 1)  # d0 = 1.0
        u.enable_input(InpSel.SRC_0, 2)  # d1 = rate
        u.require_inp0 = ENABLE
        u.repeat_count = 150
        u.trigger = (Trigger.COUNT, Trigger.NONE, Trigger.NONE)
        u.next_uop = (nxt, 0, 0)
        u.datapath_config[0] = (
            UopDpConfig()
            .enable_alu(AluOp.ADD, A.CURR_ALU_OUT, PD0)
            .pass_through_delay(1)
        )
        u.datapath_config[1] = (
            UopDpConfig()
            .enable_alu(AluOp.MULTIPLY, A.CURR_ALU_OUT, PD1)
            .enable_delay_from_src(DelayInp.PREV_ALU_OUT, 2)
        )
        u.datapath_config[2] = (
            UopDpConfig()
            .enable_alu(AluOp.IS_LT, PD2, A.CURR_SWAP_OUT)
            .enable_delay_from_src(DelayInp.PREV_ALU_OUT, 3)
            .pass_through_delay(2)
        )
        u.datapath_config[3] = (
            UopDpConfig()
            .enable_alu(AluOp.SELECT, A.CURR_ALU_OUT, PD3)
            .pass_through_delay(2, 3)
        )
        u.datapath_config[4] = (
            UopDpConfig()
            .enable_alu(AluOp.IS_LE, PD2, A.CURR_SWAP_OUT)
            .pass_through_delay(2, 3)
        )
        u.datapath_config[5] = (
            UopDpConfig()
            .enable_alu(AluOp.SELECT, A.CURR_ALU_OUT, PD3)
            .pass_through_delay(2, 3)
        )
        u.datapath_config[6] = (
            UopDpConfig()
            .enable_alu(AluOp.IS_LT, PD2, A.CURR_SWAP_OUT)
            .pass_through_delay(3)
        )
        u.datapath_config[7] = UopDpConfig().enable_alu(
            AluOp.SELECT, A.CURR_ALU_OUT, PD3
        )
        return u

    u3 = steady(4)
    u4 = steady(5)

    # --- uop5: b1: consume (bid col): emit h1@3; reseed pos@0, scan@1
    u5 = UopConfig()
    u5.enable_input(InpSel.ONE_F32, 1)  # d0 = 1.0
    u5.enable_input(InpSel.ZERO, 2)  # d1 = 0.0
    u5.require_inp0 = ENABLE
    u5.repeat_count = 1
    u5.trigger = (Trigger.COUNT, Trigger.NONE, Trigger.NONE)
    u5.next_uop = (6, 0, 0)
    u5.datapath_config[0] = (
        UopDpConfig().enable_alu(AluOp.SUBTRACT, PD1, PD0).pass_through_delay(0)
    )
    u5.datapath_config[1] = UopDpConfig().enable_alu(AluOp.BYPASS, PD0, PD0)
    u5.datapath_config[3] = UopDpConfig().enable_alu(
        AluOp.BYPASS, A.CURR_ALU_OUT, A.CURR_ALU_OUT
    )
    u5.datapath_config[4] = UopDpConfig().enable_delay_from_src(
        DelayInp.PREV_ALU_OUT, 2
    )
    u5.datapath_config[5] = UopDpConfig().pass_through_delay(2)
    u5.datapath_config[6] = UopDpConfig().pass_through_delay(2)
    u5.datapath_config[7] = UopDpConfig().pass_through_delay(2)
    u5.enable_output(OutSel.DELAY_2, OutPath.WR0_LO)

    # --- uop6: b2: consume (mp col): emit h2@5; h1=1@3
    u6 = UopConfig()
    u6.enable_input(InpSel.ONE_F32, 1)  # d0 = 1.0
    u6.require_inp0 = ENABLE
    u6.repeat_count = 1
    u6.trigger = (Trigger.COUNT, Trigger.NONE, Trigger.NONE)
    u6.next_uop = (7, 0, 0)
    u6.datapath_config[0] = UopDpConfig().pass_through_delay(0)
    u6.datapath_config[1] = UopDpConfig().pass_through_delay(0)
    u6.datapath_config[2] = UopDpConfig().pass_through_delay(0)
    u6.datapath_config[3] = UopDpConfig().enable_alu(AluOp.BYPASS, PD0, PD0)
    u6.datapath_config[5] = UopDpConfig().enable_alu(
        AluOp.BYPASS, A.CURR_ALU_OUT, A.CURR_ALU_OUT
    )
    u6.datapath_config[6] = UopDpConfig().enable_delay_from_src(
        DelayInp.PREV_ALU_OUT, 2
    )
    u6.datapath_config[7] = UopDpConfig().pass_through_delay(2)
    u6.enable_output(OutSel.DELAY_2, OutPath.WR0_LO)

    # --- uop7: b3: consume (pad): emit h3@7 (ALU_OUT); h2=1@5; loop/end
    u7 = UopConfig()
    u7.enable_input(InpSel.ONE_F32, 1)  # d0 = 1.0
    u7.require_inp0 = ENABLE
    u7.repeat_count = 1
    u7.trigger = (Trigger.SRC_TENSOR_DONE, Trigger.COUNT, Trigger.NONE)
    u7.next_uop = (0, 1, 0)
    for k in range(0, 5):
        u7.datapath_config[k] = UopDpConfig().pass_through_delay(0)
    u7.datapath_config[5] = UopDpConfig().enable_alu(AluOp.BYPASS, PD0, PD0)
    u7.datapath_config[7] = UopDpConfig().enable_alu(
        AluOp.BYPASS, A.CURR_ALU_OUT, A.CURR_ALU_OUT
    )
    u7.enable_output(OutSel.ALU_OUT, OutPath.WR0_LO)

    return [u0, u1, u2, u3, u4, u5, u6, u7]


def _ref_tap3(in0, in1, s0, s1, imm2):
    x = np.asarray(in0, np.float32)
    if x.ndim == 2:
        g = x.shape[1] // 305
        x = x.reshape(x.shape[0], g, 305)
    rates = x[:, :, 2 : 2 + S]
    cp = np.cumprod(rates, axis=-1, dtype=np.float32)
    cpz = np.concatenate(
        [np.ones((x.shape[0], x.shape[1], 1), np.float32), cp], axis=-1
    )
    idx = np.asarray(in1, np.float32).reshape(x.shape[0], x.shape[1], 2)
    bid = idx[..., 0].astype(np.int64)
    mp = idx[..., 1].astype(np.int64)
    out = np.zeros((x.shape[0], x.shape[1], 3), np.float32)
    out[..., 0] = np.take_along_axis(cpz, bid[..., None], axis=-1)[..., 0]
    out[..., 1] = np.take_along_axis(cpz, (mp + 1)[..., None], axis=-1)[..., 0]
    out[..., 2] = np.take_along_axis(cpz, mp[..., None], axis=-1)[..., 0]
    return out


def _get_ops():
    """Register the two hand-uop ops (idempotent). Returns (TAPB, TAPM)."""
    global _OPS_REGISTERED
    if _OPS_REGISTERED:
        return _OPS_REGISTERED
    import concourse.dve_ops as dve_ops
    from concourse.dve_ops import OPS
    from concourse.dve_spec import AluOp as SAluOp
    from concourse.dve_spec import Spec, Src0, Src1, scan

    for name, uops_fn, ref in (
        ("CPTAPB_ANT", _uops_tapb, _ref_tapb),
        ("CPTAPM_ANT", _uops_tapm, _ref_tapm),
        ("CPTAP3_ANT", _uops_tap3, _ref_tap3),
    ):
        if name not in dve_ops._SUB_OPCODE_FOR_NAME:
            spec = Spec(
                body=Src1 * scan(SAluOp.MULTIPLY, Src0),
                reference=ref,
            )
            op = _HandDveOp(name, spec, subdim=False, uops_fn=uops_fn)
            OPS.append(op)
            dve_ops._SUB_OPCODE_FOR_NAME[name] = (
                dve_ops._CUSTOM_DVE_ROW_BASE + len(OPS) - 1
            )
            dve_ops.CUSTOM_DVE_SPECS[name] = spec
            _OPS_REGISTERED[name] = op
        else:
            for op in OPS:
                if op.name == name:
                    _OPS_REGISTERED[name] = op
    return _OPS_REGISTERED


def build_nc(tiles=TILES, group=28):
    import concourse.bacc as bacc
    import concourse.mybir as mybir
    from concourse import tile

    f32 = mybir.dt.float32
    A = mybir.AluOpType
    ops = _get_ops()
    TAP3 = ops["CPTAP3_ANT"]

    bpc = tiles * P
    if tiles == TILES:
        # small first groups (shorter pipeline fill: first DVE op starts
        # after a ~1/4-size DMA) and a small last group (shorter tail).
        group_sizes = [14, 28, 28, 28, 28, 28, 28, 14]
    else:
        if tiles % group != 0:
            group = tiles
        group_sizes = [group] * (tiles // group)
    assert sum(group_sizes) == tiles

    nc = bacc.Bacc("TRN2", target_bir_lowering=False, debug=False)
    inp = nc.dram_tensor("inp", [bpc, COLS], f32, kind="ExternalInput")
    out = nc.dram_tensor("out", [bpc, 3], f32, kind="ExternalOutput")

    # row = p*tiles + t (partition-major) so group output DMAs coalesce
    vin = inp.ap().rearrange("(p t) c -> p t c", p=P)
    vout = out.ap().rearrange("(p t) k -> p t k", p=P)

    with tile.TileContext(nc) as tc:
        with (
            tc.tile_pool(name="raw", bufs=3) as rawp,
            tc.tile_pool(name="aux", bufs=3) as auxp,
        ):
            gmax = max(group_sizes)
            t0 = 0
            for gi, g in enumerate(group_sizes):
                rawf = rawp.tile([P, gmax, PG], f32, tag="raw")
                raw = rawf[:, 0:g, :]
                # input row -> cols 2..303 of each page; cols 0,1,304 are
                # pads (consumed but unused). Split across both HWDGE
                # queues (sync + scalar).
                nc.gpsimd.memset(raw[:, :, 0:2], 0.0)
                nc.gpsimd.memset(raw[:, :, 304:305], 0.0)
                nc.sync.dma_start(raw[0:64, :, 2 : 2 + COLS], vin[0:64, t0 : t0 + g, :])
                nc.scalar.dma_start(
                    raw[64:128, :, 2 : 2 + COLS], vin[64:128, t0 : t0 + g, :]
                )

                # contiguous (bid, mp) pairs for the flat src1 stream
                in1_f = auxp.tile([P, gmax, 2], f32, tag="in1")
                in1 = in1_f[:, 0:g, :]
                nc.gpsimd.tensor_copy(in1, raw[:, :, 302:304])

                res_f = auxp.tile([P, gmax, 3], f32, tag="res")
                res = res_f[:, 0:g, :]
                nc.vector._custom_dve(
                    TAP3,
                    out=res.rearrange("p s k -> p (s k)"),
                    in0=raw,
                    in1=in1.rearrange("p s k -> p (s k)"),
                )

                nc.sync.dma_start(vout[:, t0 : t0 + g, :], res)
                t0 += g

    nc.compile()
    return nc


_NC_CACHE = {}


def _get_nc():
    key = (TILES, 28)
    if key not in _NC_CACHE:
        _NC_CACHE[key] = build_nc()
    return _NC_CACHE[key]


def kernel(inputs):
    global LAST_RESULTS
    x = np.ascontiguousarray(np.asarray(inputs), dtype=np.float32)
    assert x.shape == (BTOT, COLS), x.shape

    npad = BPC * NCORES - BTOT
    padrows = np.zeros((npad, COLS), dtype=np.float32)
    padrows[:, :S] = 1.0
    xp = np.concatenate([x, padrows], axis=0)
    shards = xp.reshape(NCORES, BPC, COLS)

    in_maps = [{"inp": np.ascontiguousarray(shards[c])} for c in range(NCORES)]

    nc = _get_nc()
    from concourse.bass_utils import run_bass_kernel_spmd

    r = run_bass_kernel_spmd(
        nc, in_maps, core_ids=list(range(NCORES)), trace=TRACE
    )
    LAST_RESULTS = r
    y = np.concatenate([r.results[c]["out"] for c in range(NCORES)], axis=0)
    return np.ascontiguousarray(y[:BTOT]).astype(np.float32)
